# revision 14
# baseline (speedup 1.0000x reference)
"""Multi-head self-attention Trainium2 kernel (Bass/Tile), 8-core SPMD.

Problem: x[8, 2048, 384], 6 heads x 64 dim, torch-style qkv/out projections.
Sharding: pure data-parallel over batch B=8 -> one batch per NeuronCore,
full weights broadcast to every core; no collectives needed.

Per-core pipeline (everything stays on-chip; only x-slice, weights in and
out-slice leave HBM):
  1. PE-transpose w_qkv -> WqkvT [d,e], w_out -> WoutT [d,e], x -> xT [d,n].
  2. QKV projection qkvT[e,n] = WqkvT.T @ xT (+bias; Q pre-scaled by 1/8),
     laid out so each 128-partition tile holds a pair of heads (64+64).
  3. Attention per head-pair: S^T tiles = K_j stationary vs Q^T moving --
     two heads run concurrently on disjoint PE row groups; exp on ScalarE;
     AV via lhsT=[V|ones] so the softmax denominator comes out as a spare
     PSUM row of the U'^T accumulation (no extra pass).
  4. Denominator: broadcast-DMA the denom rows across partitions, one DVE
     reciprocal + one DVE multiply normalizes attnT.
  5. Output projection y = attnT.T @ WoutT + b_out, DMA out.
Matmuls use float32r (full-rate fp32 mode, exact in sim; ~tf32-ish on HW).
"""

import numpy as np
from contextlib import ExitStack

import concourse.bass as bass
import concourse.mybir as mybir
import concourse.tile as tile
from concourse import bacc
from concourse.bass_utils import run_bass_kernel_spmd
from concourse.masks import make_identity

F32 = mybir.dt.float32
F32R = mybir.dt.float32r

# Problem dims (hardcoded per contract)
B = 8
N = 2048
D = 384
H = 6
DH = 64
E3 = 3 * D  # 1152
SCALE = DH ** -0.5  # 0.125

P = 128
NT = N // P    # 16 tiles along sequence
DT = D // P    # 3 tiles along embed
ET = E3 // P   # 9 tiles along qkv-out
NPAIR = H // 2  # 3 head pairs
CHUNK = 1024   # n-chunk for attention inner loop (2 PSUM banks)
NCH = N // CHUNK


def build_module(debug=False):
    nc = bacc.Bacc(None)
    x_ext = nc.declare_dram_parameter("x", [N, D], F32, isOutput=False)
    wqkv_ext = nc.declare_dram_parameter("w_qkv", [E3, D], F32, isOutput=False)
    bqkv_ext = nc.declare_dram_parameter("b_qkv", [E3], F32, isOutput=False)
    wout_ext = nc.declare_dram_parameter("w_out", [D, D], F32, isOutput=False)
    bout_ext = nc.declare_dram_parameter("b_out", [D], F32, isOutput=False)
    out_ext = nc.declare_dram_parameter("out", [N, D], F32, isOutput=True)
    if debug:
        dbg_qkvT = nc.declare_dram_parameter("dbg_qkvT", [P, ET, N], F32, isOutput=True)
        dbg_vnat = nc.declare_dram_parameter("dbg_vnat", [P, NPAIR, NT, 130], F32, isOutput=True)
        dbg_expA = nc.declare_dram_parameter("dbg_expA", [P, CHUNK], F32, isOutput=True)
        dbg_uA = nc.declare_dram_parameter("dbg_uA", [P, CHUNK], F32, isOutput=True)
        dbg_recip = nc.declare_dram_parameter("dbg_recip", [P, N], F32, isOutput=True)
        dbg_attnT = nc.declare_dram_parameter("dbg_attnT", [P, DT, N], F32, isOutput=True)

    with ExitStack() as stack:
        tc = stack.enter_context(tile.TileContext(nc))
        persist = stack.enter_context(tc.tile_pool(name="persist", bufs=1))
        ps = stack.enter_context(tc.tile_pool(name="ps", bufs=2, space="PSUM"))
        psu = stack.enter_context(tc.tile_pool(name="psu", bufs=2, space="PSUM"))

        identity = persist.tile([P, P], F32)
        make_identity(nc, identity)
        onesf = persist.tile([P, P], F32)
        nc.gpsimd.memset(onesf[:], 1.0)

        qkvT = persist.tile([P, ET, N], F32R)     # [dp, e-tile, n]; e-tiles 0-2=Q,3-5=K,6-8=V
        attnT = persist.tile([P, DT, N], F32R)    # unnormalized U^T, heads stacked
        vnat = persist.tile([P, NPAIR, NT, 130], F32R)  # [m-part, pair, m-tile, V_A|1|V_B|1]
        woutT = persist.tile([P, DT, D], F32R)
        bout_rep = persist.tile([P, D], F32)
        bqkv_sb = persist.tile([P, ET], F32)

        # ---------------- Stage A/B: loads + PE transposes ----------------
        with tc.tile_pool(name="early", bufs=1) as early:
            wqkvT = early.tile([P, DT, E3], F32R)
            xT = early.tile([P, DT, N], F32R)

            nc.sync.dma_start(out=bqkv_sb[:], in_=bqkv_ext[:].rearrange("(o p) -> p o", p=P))
            btmp = early.tile([1, D], F32)
            nc.sync.dma_start(out=btmp[:], in_=bout_ext[:].rearrange("d -> () d"))
            pbx = ps.tile([P, P], F32, tag="ps")
            nc.tensor.matmul(pbx[:, 0:D // DT], lhsT=onesf[0:1, :], rhs=btmp[0:1, 0:D // DT], start=True, stop=True)
            nc.vector.tensor_copy(out=bout_rep[:, 0:D // DT], in_=pbx[:, 0:D // DT])
            pbx2 = ps.tile([P, P], F32, tag="ps")
            nc.tensor.matmul(pbx2[:], lhsT=onesf[0:1, :], rhs=btmp[0:1, D // DT:D // DT + P], start=True, stop=True)
            nc.vector.tensor_copy(out=bout_rep[:, D // DT:D // DT + P], in_=pbx2[:])
            pbx3 = ps.tile([P, P], F32, tag="ps")
            nc.tensor.matmul(pbx3[:], lhsT=onesf[0:1, :], rhs=btmp[0:1, D // DT + P:D], start=True, stop=True)
            nc.vector.tensor_copy(out=bout_rep[:, D // DT + P:D], in_=pbx3[:])

            for eo in range(ET):
                wt = early.tile([P, D], F32, tag="wload", bufs=3)
                nc.sync.dma_start(out=wt[:], in_=wqkv_ext[eo * P:(eo + 1) * P, :])
                for do in range(DT):
                    pt = ps.tile([P, P], F32, tag="ps")
                    nc.tensor.transpose(pt[:], wt[:, do * P:(do + 1) * P], identity)
                    nc.vector.tensor_copy(out=wqkvT[:, do, eo * P:(eo + 1) * P], in_=pt[:])

            for eo in range(DT):
                wt = early.tile([P, D], F32, tag="wload", bufs=3)
                nc.sync.dma_start(out=wt[:], in_=wout_ext[eo * P:(eo + 1) * P, :])
                for do in range(DT):
                    pt = ps.tile([P, P], F32, tag="ps")
                    nc.tensor.transpose(pt[:], wt[:, do * P:(do + 1) * P], identity)
                    nc.vector.tensor_copy(out=woutT[:, do, eo * P:(eo + 1) * P], in_=pt[:])

            for no in range(NT):
                xt = early.tile([P, D], F32, tag="xload", bufs=3)
                nc.sync.dma_start(out=xt[:], in_=x_ext[no * P:(no + 1) * P, :])
                for do in range(DT):
                    pt = ps.tile([P, P], F32, tag="ps")
                    nc.tensor.transpose(pt[:], xt[:, do * P:(do + 1) * P], identity)
                    nc.vector.tensor_copy(out=xT[:, do, no * P:(no + 1) * P], in_=pt[:])

            # ---------------- Stage C: QKV projection ----------------
            # qkvT[:, eo, n] = WqkvT[:, :, e-tile].T @ xT[:, :, n] (+ bias, Q scaled)
            for eo in range(ET):
                for c in range(NCH):
                    pq = ps.tile([P, CHUNK], F32, tag="ps")
                    for h2 in range(CHUNK // 512):
                        lo = h2 * 512
                        for do in range(DT):
                            nc.tensor.matmul(
                                pq[:, lo:lo + 512],
                                lhsT=wqkvT[:, do, eo * P:(eo + 1) * P],
                                rhs=xT[:, do, c * CHUNK + lo:c * CHUNK + lo + 512],
                                start=(do == 0),
                                stop=(do == DT - 1),
                            )
                    dst = qkvT[:, eo, c * CHUNK:(c + 1) * CHUNK]
                    bcol = bqkv_sb[:, eo:eo + 1]
                    if eo < 3:  # Q: (x@Wq + bq) * SCALE
                        nc.vector.tensor_scalar(
                            dst, pq[:], bcol, SCALE,
                            mybir.AluOpType.add, mybir.AluOpType.mult,
                        )
                    else:
                        nc.vector.tensor_scalar_add(dst, pq[:], bcol)

        if debug:
            nc.sync.dma_start(out=dbg_qkvT[:], in_=qkvT[:].bitcast(F32))

        # ---------------- V natural layout (+ones col) for AV ----------------
        nc.gpsimd.memset(vnat[:, :, :, 64:65].bitcast(F32), 1.0)
        nc.gpsimd.memset(vnat[:, :, :, 129:130].bitcast(F32), 1.0)
        for pr in range(NPAIR):
            for j in range(NT):
                pt = ps.tile([P, P], F32, tag="ps")
                nc.tensor.transpose(pt[:], qkvT[:, 6 + pr, j * P:(j + 1) * P].bitcast(F32), identity)
                nc.vector.tensor_copy(out=vnat[:, pr, j, 0:64], in_=pt[:, 0:64])
                nc.vector.tensor_copy(out=vnat[:, pr, j, 65:129], in_=pt[:, 64:128])

        if debug:
            nc.sync.dma_start(out=dbg_vnat[:], in_=vnat[:].bitcast(F32))

        # ---------------- Stage D: attention per head pair ----------------
        with tc.tile_pool(name="attnp", bufs=1) as attnp:
            for pr in range(NPAIR):
                recipP = attnp.tile([P, N], F32, tag="recipP", bufs=2)
                for c in range(NCH):
                    cs = slice(c * CHUNK, (c + 1) * CHUNK)
                    uA = psu.tile([P, CHUNK], F32, tag="psu")
                    uB = psu.tile([P, CHUNK], F32, tag="psu")
                    for j in range(NT):
                        sA = ps.tile([P, CHUNK], F32, tag="ps")
                        sB = ps.tile([P, CHUNK], F32, tag="ps")
                        for h2 in range(CHUNK // 512):
                            lo = h2 * 512
                            qs = slice(c * CHUNK + lo, c * CHUNK + lo + 512)
                            nc.tensor.matmul(
                                sA[:, lo:lo + 512],
                                lhsT=qkvT[0:64, 3 + pr, j * P:(j + 1) * P],
                                rhs=qkvT[0:64, pr, qs],
                                start=True, stop=True,
                            )
                            nc.tensor.matmul(
                                sB[:, lo:lo + 512],
                                lhsT=qkvT[64:128, 3 + pr, j * P:(j + 1) * P],
                                rhs=qkvT[64:128, pr, qs],
                                start=True, stop=True,
                            )
                        eA = attnp.tile([P, CHUNK], F32R, tag="exp", bufs=4)
                        eB = attnp.tile([P, CHUNK], F32R, tag="exp", bufs=4)
                        nc.scalar.activation(eA[:], sA[:], mybir.ActivationFunctionType.Exp)
                        if debug and pr == 0 and c == 0 and j == 0:
                            nc.sync.dma_start(out=dbg_expA[:], in_=eA[:].bitcast(F32))
                        nc.scalar.activation(eB[:], sB[:], mybir.ActivationFunctionType.Exp)
                        for h2 in range(CHUNK // 512):
                            lo = h2 * 512
                            nc.tensor.matmul(
                                uA[0:65, lo:lo + 512],
                                lhsT=vnat[:, pr, j, 0:65],
                                rhs=eA[:, lo:lo + 512],
                                start=(j == 0), stop=(j == NT - 1),
                            )
                            nc.tensor.matmul(
                                uB[0:65, lo:lo + 512],
                                lhsT=vnat[:, pr, j, 65:130],
                                rhs=eB[:, lo:lo + 512],
                                start=(j == 0), stop=(j == NT - 1),
                            )
                    # U'^T rows: u[0:64]=U_head, u[64]=softmax denominator.
                    # DMA cannot read PSUM, so stage through SBUF via DVE,
                    # then DMA handles the partition placement/broadcast.
                    stA = attnp.tile([P, CHUNK], F32R, tag="stU", bufs=2)
                    stB = attnp.tile([P, CHUNK], F32R, tag="stU", bufs=2)
                    nc.vector.tensor_copy(out=stA[0:65, :], in_=uA[0:65, :])
                    nc.vector.tensor_copy(out=stB[0:65, :], in_=uB[0:65, :])
                    if debug and pr == 0 and c == 0:
                        nc.sync.dma_start(out=dbg_uA[:], in_=stA[:].bitcast(F32))
                    nc.sync.dma_start(out=attnT[0:64, pr, cs], in_=stA[0:64, :])
                    nc.sync.dma_start(out=attnT[64:128, pr, cs], in_=stB[0:64, :])
                    pbb = psu.tile([P, CHUNK], F32, tag="psu")
                    for h2 in range(CHUNK // 512):
                        lo = h2 * 512
                        nc.tensor.matmul(pbb[0:64, lo:lo + 512], lhsT=onesf[64:65, 0:64],
                                         rhs=stA[64:65, lo:lo + 512].bitcast(F32),
                                         start=True, stop=True)
                        nc.tensor.matmul(pbb[64:128, lo:lo + 512], lhsT=onesf[64:65, 0:64],
                                         rhs=stB[64:65, lo:lo + 512].bitcast(F32),
                                         start=True, stop=True)
                    nc.vector.reciprocal(recipP[:, cs], pbb[:])
                nc.vector.tensor_mul(attnT[:, pr, :], attnT[:, pr, :], recipP[:])
                if debug and pr == 0:
                    nc.sync.dma_start(out=dbg_recip[:], in_=recipP[:])

            if debug:
                nc.sync.dma_start(out=dbg_attnT[:], in_=attnT[:].bitcast(F32))

            # ---------------- Stage E: output projection ----------------
            for i in range(NT):
                py = ps.tile([P, D], F32, tag="ps")
                for do in range(DT):
                    nc.tensor.matmul(
                        py[:],
                        lhsT=attnT[:, do, i * P:(i + 1) * P],
                        rhs=woutT[:, do, :],
                        start=(do == 0), stop=(do == DT - 1),
                    )
                yt = attnp.tile([P, D], F32, tag="yt", bufs=3)
                nc.vector.tensor_add(yt[:], py[:], bout_rep[:])
                nc.sync.dma_start(out=out_ext[i * P:(i + 1) * P, :], in_=yt[:])

    nc.finalize()
    return nc


_NC_CACHE = None


def _get_nc():
    global _NC_CACHE
    if _NC_CACHE is None:
        _NC_CACHE = build_module()
    return _NC_CACHE


def run(x, w_qkv, b_qkv, w_out, b_out, **spmd_kwargs):
    x = np.ascontiguousarray(np.asarray(x, dtype=np.float32))
    w_qkv = np.ascontiguousarray(np.asarray(w_qkv, dtype=np.float32))
    b_qkv = np.ascontiguousarray(np.asarray(b_qkv, dtype=np.float32))
    w_out = np.ascontiguousarray(np.asarray(w_out, dtype=np.float32))
    b_out = np.ascontiguousarray(np.asarray(b_out, dtype=np.float32))

    nc = _get_nc()
    core_ids = list(range(B))
    in_maps = [
        {"x": x[b], "w_qkv": w_qkv, "b_qkv": b_qkv, "w_out": w_out, "b_out": b_out}
        for b in range(B)
    ]
    res = run_bass_kernel_spmd(nc, in_maps, core_ids, **spmd_kwargs)
    out = np.stack([res.results[b]["out"] for b in range(B)], axis=0)
    return out, res


def kernel(x, w_qkv, b_qkv, w_out, b_out):
    out, _ = run(x, w_qkv, b_qkv, w_out, b_out)
    return out


# revision 15
# speedup vs baseline: 13.4636x; 13.4636x over previous
"""Multi-head self-attention Trainium2 kernel (Bass/Tile), 8-core SPMD.

Problem: x[8, 2048, 384], 6 heads x 64 dim, torch-style qkv/out projections.
Sharding: pure data-parallel over batch B=8 -> one batch per NeuronCore,
full weights broadcast to every core; no collectives needed.

Per-core pipeline (everything stays on-chip; only x-slice, weights in and
out-slice leave HBM):
  1. PE-transpose w_qkv -> WqkvT [d,e], w_out -> WoutT [d,e], x -> xT [d,n].
  2. QKV projection qkvT[e,n] = WqkvT.T @ xT (+bias; Q pre-scaled by 1/8),
     laid out so each 128-partition tile holds a pair of heads (64+64).
  3. Attention per head-pair: S^T tiles = K_j stationary vs Q^T moving --
     two heads run concurrently on disjoint PE row groups; exp on ScalarE;
     AV via lhsT=[V|ones] so the softmax denominator comes out as a spare
     PSUM row of the U'^T accumulation (no extra pass).
  4. Denominator: broadcast-DMA the denom rows across partitions, one DVE
     reciprocal + one DVE multiply normalizes attnT.
  5. Output projection y = attnT.T @ WoutT + b_out, DMA out.
Matmuls use float32r (full-rate fp32 mode, exact in sim; ~tf32-ish on HW).
"""

import numpy as np
from contextlib import ExitStack

import concourse.bass as bass
import concourse.mybir as mybir
import concourse.tile as tile
from concourse import bacc
from concourse.bass_utils import run_bass_kernel_spmd
from concourse.masks import make_identity

F32 = mybir.dt.float32
F32R = mybir.dt.float32r

# Problem dims (hardcoded per contract)
B = 8
N = 2048
D = 384
H = 6
DH = 64
E3 = 3 * D  # 1152
SCALE = DH ** -0.5  # 0.125

P = 128
NT = N // P    # 16 tiles along sequence
DT = D // P    # 3 tiles along embed
ET = E3 // P   # 9 tiles along qkv-out
NPAIR = H // 2  # 3 head pairs
CHUNK = 1024   # n-chunk for attention inner loop (2 PSUM banks)
NCH = N // CHUNK


def build_module(debug=False, reps=1):
    nc = bacc.Bacc(None)
    x_ext = nc.declare_dram_parameter("x", [N, D], F32, isOutput=False)
    wqkv_ext = nc.declare_dram_parameter("w_qkv", [E3, D], F32, isOutput=False)
    bqkv_ext = nc.declare_dram_parameter("b_qkv", [E3], F32, isOutput=False)
    wout_ext = nc.declare_dram_parameter("w_out", [D, D], F32, isOutput=False)
    bout_ext = nc.declare_dram_parameter("b_out", [D], F32, isOutput=False)
    out_ext = nc.declare_dram_parameter("out", [N, D], F32, isOutput=True)
    if debug:
        dbg_qkvT = nc.declare_dram_parameter("dbg_qkvT", [P, ET, N], F32, isOutput=True)
        dbg_vnat = nc.declare_dram_parameter("dbg_vnat", [P, NPAIR, NT, 130], F32, isOutput=True)
        dbg_expA = nc.declare_dram_parameter("dbg_expA", [P, CHUNK], F32, isOutput=True)
        dbg_uA = nc.declare_dram_parameter("dbg_uA", [P, CHUNK], F32, isOutput=True)
        dbg_recip = nc.declare_dram_parameter("dbg_recip", [P, N], F32, isOutput=True)
        dbg_attnT = nc.declare_dram_parameter("dbg_attnT", [P, DT, N], F32, isOutput=True)

    with ExitStack() as stack:
        tc = stack.enter_context(tile.TileContext(nc))
        persist = stack.enter_context(tc.tile_pool(name="persist", bufs=1))
        ps = stack.enter_context(tc.tile_pool(name="ps", bufs=2, space="PSUM"))
        psu = stack.enter_context(tc.tile_pool(name="psu", bufs=2, space="PSUM"))

        identity = persist.tile([P, P], F32)
        make_identity(nc, identity)
        onesf = persist.tile([P, P], F32)
        nc.gpsimd.memset(onesf[:], 1.0)

        rep_ctx = tc.For_i(0, reps, 1) if reps > 1 else None
        if rep_ctx is not None:
            stack.enter_context(rep_ctx)

        qkvT = persist.tile([P, ET, N], F32R)     # [dp, e-tile, n]; e-tiles 0-2=Q,3-5=K,6-8=V
        attnT = persist.tile([P, DT, N], F32R)    # unnormalized U^T, heads stacked
        vnat = persist.tile([P, NPAIR, NT, 130], F32R)  # [m-part, pair, m-tile, V_A|1|V_B|1]
        woutT = persist.tile([P, DT, D], F32R)
        bout_rep = persist.tile([P, D], F32)
        bqkv_sb = persist.tile([P, ET], F32)

        # ---------------- Stage A/B: loads + PE transposes ----------------
        with tc.tile_pool(name="early", bufs=1) as early:
            wqkvT = early.tile([P, DT, E3], F32R)
            xT = early.tile([P, DT, N], F32R)

            nc.sync.dma_start(out=bqkv_sb[:], in_=bqkv_ext[:].rearrange("(o p) -> p o", p=P))
            btmp = early.tile([1, D], F32)
            nc.sync.dma_start(out=btmp[:], in_=bout_ext[:].rearrange("d -> () d"))
            pbx = ps.tile([P, P], F32, tag="ps")
            nc.tensor.matmul(pbx[:, 0:D // DT], lhsT=onesf[0:1, :], rhs=btmp[0:1, 0:D // DT], start=True, stop=True)
            nc.vector.tensor_copy(out=bout_rep[:, 0:D // DT], in_=pbx[:, 0:D // DT])
            pbx2 = ps.tile([P, P], F32, tag="ps")
            nc.tensor.matmul(pbx2[:], lhsT=onesf[0:1, :], rhs=btmp[0:1, D // DT:D // DT + P], start=True, stop=True)
            nc.vector.tensor_copy(out=bout_rep[:, D // DT:D // DT + P], in_=pbx2[:])
            pbx3 = ps.tile([P, P], F32, tag="ps")
            nc.tensor.matmul(pbx3[:], lhsT=onesf[0:1, :], rhs=btmp[0:1, D // DT + P:D], start=True, stop=True)
            nc.vector.tensor_copy(out=bout_rep[:, D // DT + P:D], in_=pbx3[:])

            for eo in range(ET):
                wt = early.tile([P, D], F32, tag="wload", bufs=3)
                nc.sync.dma_start(out=wt[:], in_=wqkv_ext[eo * P:(eo + 1) * P, :])
                for do in range(DT):
                    pt = ps.tile([P, P], F32, tag="ps")
                    nc.tensor.transpose(pt[:], wt[:, do * P:(do + 1) * P], identity)
                    nc.vector.tensor_copy(out=wqkvT[:, do, eo * P:(eo + 1) * P], in_=pt[:])

            for eo in range(DT):
                wt = early.tile([P, D], F32, tag="wload", bufs=3)
                nc.sync.dma_start(out=wt[:], in_=wout_ext[eo * P:(eo + 1) * P, :])
                for do in range(DT):
                    pt = ps.tile([P, P], F32, tag="ps")
                    nc.tensor.transpose(pt[:], wt[:, do * P:(do + 1) * P], identity)
                    nc.vector.tensor_copy(out=woutT[:, do, eo * P:(eo + 1) * P], in_=pt[:])

            for no in range(NT):
                xt = early.tile([P, D], F32, tag="xload", bufs=3)
                nc.sync.dma_start(out=xt[:], in_=x_ext[no * P:(no + 1) * P, :])
                for do in range(DT):
                    pt = ps.tile([P, P], F32, tag="ps")
                    nc.tensor.transpose(pt[:], xt[:, do * P:(do + 1) * P], identity)
                    nc.vector.tensor_copy(out=xT[:, do, no * P:(no + 1) * P], in_=pt[:])

            # ---------------- Stage C: QKV projection ----------------
            # qkvT[:, eo, n] = WqkvT[:, :, e-tile].T @ xT[:, :, n] (+ bias, Q scaled)
            for eo in range(ET):
                for c in range(NCH):
                    pq = ps.tile([P, CHUNK], F32, tag="ps")
                    for h2 in range(CHUNK // 512):
                        lo = h2 * 512
                        for do in range(DT):
                            nc.tensor.matmul(
                                pq[:, lo:lo + 512],
                                lhsT=wqkvT[:, do, eo * P:(eo + 1) * P],
                                rhs=xT[:, do, c * CHUNK + lo:c * CHUNK + lo + 512],
                                start=(do == 0),
                                stop=(do == DT - 1),
                            )
                    dst = qkvT[:, eo, c * CHUNK:(c + 1) * CHUNK]
                    bcol = bqkv_sb[:, eo:eo + 1]
                    if eo < 3:  # Q: (x@Wq + bq) * SCALE
                        nc.vector.tensor_scalar(
                            dst, pq[:], bcol, SCALE,
                            mybir.AluOpType.add, mybir.AluOpType.mult,
                        )
                    else:
                        nc.vector.tensor_scalar_add(dst, pq[:], bcol)

        if debug:
            nc.sync.dma_start(out=dbg_qkvT[:], in_=qkvT[:].bitcast(F32))

        # ---------------- V natural layout (+ones col) for AV ----------------
        nc.gpsimd.memset(vnat[:, :, :, 64:65].bitcast(F32), 1.0)
        nc.gpsimd.memset(vnat[:, :, :, 129:130].bitcast(F32), 1.0)
        for pr in range(NPAIR):
            for j in range(NT):
                pt = ps.tile([P, P], F32, tag="ps")
                nc.tensor.transpose(pt[:], qkvT[:, 6 + pr, j * P:(j + 1) * P].bitcast(F32), identity)
                nc.vector.tensor_copy(out=vnat[:, pr, j, 0:64], in_=pt[:, 0:64])
                nc.vector.tensor_copy(out=vnat[:, pr, j, 65:129], in_=pt[:, 64:128])

        if debug:
            nc.sync.dma_start(out=dbg_vnat[:], in_=vnat[:].bitcast(F32))

        # ---------------- Stage D: attention per head pair ----------------
        with tc.tile_pool(name="attnp", bufs=1) as attnp:
            for pr in range(NPAIR):
                recipP = attnp.tile([P, N], F32, tag="recipP", bufs=2)
                for c in range(NCH):
                    cs = slice(c * CHUNK, (c + 1) * CHUNK)
                    uA = psu.tile([P, CHUNK], F32, tag="psu")
                    uB = psu.tile([P, CHUNK], F32, tag="psu")
                    for j in range(NT):
                        sA = ps.tile([P, CHUNK], F32, tag="ps")
                        sB = ps.tile([P, CHUNK], F32, tag="ps")
                        for h2 in range(CHUNK // 512):
                            lo = h2 * 512
                            qs = slice(c * CHUNK + lo, c * CHUNK + lo + 512)
                            nc.tensor.matmul(
                                sA[:, lo:lo + 512],
                                lhsT=qkvT[0:64, 3 + pr, j * P:(j + 1) * P],
                                rhs=qkvT[0:64, pr, qs],
                                start=True, stop=True,
                            )
                            nc.tensor.matmul(
                                sB[:, lo:lo + 512],
                                lhsT=qkvT[64:128, 3 + pr, j * P:(j + 1) * P],
                                rhs=qkvT[64:128, pr, qs],
                                start=True, stop=True,
                            )
                        eA = attnp.tile([P, CHUNK], F32R, tag="exp", bufs=4)
                        eB = attnp.tile([P, CHUNK], F32R, tag="exp", bufs=4)
                        nc.scalar.activation(eA[:], sA[:], mybir.ActivationFunctionType.Exp)
                        if debug and pr == 0 and c == 0 and j == 0:
                            nc.sync.dma_start(out=dbg_expA[:], in_=eA[:].bitcast(F32))
                        nc.scalar.activation(eB[:], sB[:], mybir.ActivationFunctionType.Exp)
                        for h2 in range(CHUNK // 512):
                            lo = h2 * 512
                            nc.tensor.matmul(
                                uA[0:65, lo:lo + 512],
                                lhsT=vnat[:, pr, j, 0:65],
                                rhs=eA[:, lo:lo + 512],
                                start=(j == 0), stop=(j == NT - 1),
                            )
                            nc.tensor.matmul(
                                uB[0:65, lo:lo + 512],
                                lhsT=vnat[:, pr, j, 65:130],
                                rhs=eB[:, lo:lo + 512],
                                start=(j == 0), stop=(j == NT - 1),
                            )
                    # U'^T rows: u[0:64]=U_head, u[64]=softmax denominator.
                    # DMA cannot read PSUM, so stage through SBUF via DVE,
                    # then DMA handles the partition placement/broadcast.
                    stA = attnp.tile([P, CHUNK], F32R, tag="stU", bufs=2)
                    stB = attnp.tile([P, CHUNK], F32R, tag="stU", bufs=2)
                    nc.vector.tensor_copy(out=stA[0:65, :], in_=uA[0:65, :])
                    nc.vector.tensor_copy(out=stB[0:65, :], in_=uB[0:65, :])
                    if debug and pr == 0 and c == 0:
                        nc.sync.dma_start(out=dbg_uA[:], in_=stA[:].bitcast(F32))
                    nc.sync.dma_start(out=attnT[0:64, pr, cs], in_=stA[0:64, :])
                    nc.sync.dma_start(out=attnT[64:128, pr, cs], in_=stB[0:64, :])
                    pbb = psu.tile([P, CHUNK], F32, tag="psu")
                    for h2 in range(CHUNK // 512):
                        lo = h2 * 512
                        nc.tensor.matmul(pbb[0:64, lo:lo + 512], lhsT=onesf[64:65, 0:64],
                                         rhs=stA[64:65, lo:lo + 512].bitcast(F32),
                                         start=True, stop=True)
                        nc.tensor.matmul(pbb[64:128, lo:lo + 512], lhsT=onesf[64:65, 0:64],
                                         rhs=stB[64:65, lo:lo + 512].bitcast(F32),
                                         start=True, stop=True)
                    nc.vector.reciprocal(recipP[:, cs], pbb[:])
                nc.vector.tensor_mul(attnT[:, pr, :], attnT[:, pr, :], recipP[:])
                if debug and pr == 0:
                    nc.sync.dma_start(out=dbg_recip[:], in_=recipP[:])

            if debug:
                nc.sync.dma_start(out=dbg_attnT[:], in_=attnT[:].bitcast(F32))

            # ---------------- Stage E: output projection ----------------
            for i in range(NT):
                py = ps.tile([P, D], F32, tag="ps")
                for do in range(DT):
                    nc.tensor.matmul(
                        py[:],
                        lhsT=attnT[:, do, i * P:(i + 1) * P],
                        rhs=woutT[:, do, :],
                        start=(do == 0), stop=(do == DT - 1),
                    )
                yt = attnp.tile([P, D], F32, tag="yt", bufs=3)
                nc.vector.tensor_add(yt[:], py[:], bout_rep[:])
                nc.sync.dma_start(out=out_ext[i * P:(i + 1) * P, :], in_=yt[:])

    nc.finalize()
    return nc


_NC_CACHE = None


def _get_nc():
    global _NC_CACHE
    if _NC_CACHE is None:
        _NC_CACHE = build_module()
    return _NC_CACHE


def run(x, w_qkv, b_qkv, w_out, b_out, **spmd_kwargs):
    x = np.ascontiguousarray(np.asarray(x, dtype=np.float32))
    w_qkv = np.ascontiguousarray(np.asarray(w_qkv, dtype=np.float32))
    b_qkv = np.ascontiguousarray(np.asarray(b_qkv, dtype=np.float32))
    w_out = np.ascontiguousarray(np.asarray(w_out, dtype=np.float32))
    b_out = np.ascontiguousarray(np.asarray(b_out, dtype=np.float32))

    nc = _get_nc()
    core_ids = list(range(B))
    in_maps = [
        {"x": x[b], "w_qkv": w_qkv, "b_qkv": b_qkv, "w_out": w_out, "b_out": b_out}
        for b in range(B)
    ]
    res = run_bass_kernel_spmd(nc, in_maps, core_ids, **spmd_kwargs)
    out = np.stack([res.results[b]["out"] for b in range(B)], axis=0)
    return out, res


def kernel(x, w_qkv, b_qkv, w_out, b_out):
    out, _ = run(x, w_qkv, b_qkv, w_out, b_out)
    return out


# revision 20
# speedup vs baseline: 13.6022x; 1.0103x over previous
"""Multi-head self-attention Trainium2 kernel (Bass/Tile), 8-core SPMD.

Problem: x[8, 2048, 384], 6 heads x 64 dim, torch-style qkv/out projections.
Sharding: pure data-parallel over batch B=8 -> one batch per NeuronCore,
full weights broadcast to every core; no collectives needed.

Per-core pipeline (everything stays on-chip; only x-slice, weights in and
out-slice leave HBM):
  1. PE-transpose w_qkv -> WqkvT [d,e], w_out -> WoutT [d,e], x -> xT [d,n].
  2. QKV projection qkvT[e,n] = WqkvT.T @ xT (+bias; Q pre-scaled by 1/8),
     laid out so each 128-partition tile holds a pair of heads (64+64).
  3. Attention per head-pair: S^T tiles = K_j stationary vs Q^T moving --
     two heads run concurrently on disjoint PE row groups; exp on ScalarE;
     AV via lhsT=[V|ones] so the softmax denominator comes out as a spare
     PSUM row of the U'^T accumulation (no extra pass).  512-wide chunks
     keep S tiles at 1 PSUM bank so they can triple-buffer (ps pool bufs=6)
     and the PE runs ahead of ScalarE instead of serializing on it.
  4. Denominator: replicate the denom row across partitions with a K=1
     ones-matmul, one DVE reciprocal + one DVE multiply normalizes attnT.
  5. Output projection y = attnT.T @ WoutT + b_out, DMA out.
Matmuls use float32r (full-rate fp32 mode, exact in sim; ~tf32 on HW).
"""

import numpy as np
from contextlib import ExitStack

import concourse.bass as bass
import concourse.mybir as mybir
import concourse.tile as tile
from concourse import bacc
from concourse.bass_utils import run_bass_kernel_spmd
from concourse.masks import make_identity

F32 = mybir.dt.float32
F32R = mybir.dt.float32r

# Problem dims (hardcoded per contract)
B = 8
N = 2048
D = 384
H = 6
DH = 64
E3 = 3 * D  # 1152
SCALE = DH ** -0.5  # 0.125

P = 128
NT = N // P     # 16 tiles along sequence
DT = D // P     # 3 tiles along embed
ET = E3 // P    # 9 tiles along qkv-out
NPAIR = H // 2  # 3 head pairs
SCH = 512       # attention n-chunk: 1 PSUM bank
NSC = N // SCH  # 4 chunks


def build_module(debug=False, reps=1, ablate=()):
    nc = bacc.Bacc(None)
    x_ext = nc.declare_dram_parameter("x", [N, D], F32, isOutput=False)
    wqkv_ext = nc.declare_dram_parameter("w_qkv", [E3, D], F32, isOutput=False)
    bqkv_ext = nc.declare_dram_parameter("b_qkv", [E3], F32, isOutput=False)
    wout_ext = nc.declare_dram_parameter("w_out", [D, D], F32, isOutput=False)
    bout_ext = nc.declare_dram_parameter("b_out", [D], F32, isOutput=False)
    out_ext = nc.declare_dram_parameter("out", [N, D], F32, isOutput=True)
    if debug:
        dbg_qkvT = nc.declare_dram_parameter("dbg_qkvT", [P, ET, N], F32, isOutput=True)
        dbg_vnat = nc.declare_dram_parameter("dbg_vnat", [P, NPAIR, NT, 130], F32, isOutput=True)
        dbg_expA = nc.declare_dram_parameter("dbg_expA", [P, SCH], F32, isOutput=True)
        dbg_uA = nc.declare_dram_parameter("dbg_uA", [P, SCH], F32, isOutput=True)
        dbg_recip = nc.declare_dram_parameter("dbg_recip", [P, N], F32, isOutput=True)
        dbg_attnT = nc.declare_dram_parameter("dbg_attnT", [P, DT, N], F32, isOutput=True)

    with ExitStack() as stack:
        tc = stack.enter_context(tile.TileContext(nc))
        persist = stack.enter_context(tc.tile_pool(name="persist", bufs=1))
        # PSUM: "ps" 6 tags-shared 1-bank slots + "psu" 2 accumulators = 8 banks
        ps = stack.enter_context(tc.tile_pool(name="ps", bufs=6, space="PSUM"))
        psu = stack.enter_context(tc.tile_pool(name="psu", bufs=2, space="PSUM"))

        identity = persist.tile([P, P], F32)
        make_identity(nc, identity)
        onesf = persist.tile([P, P], F32)
        nc.gpsimd.memset(onesf[:], 1.0)
        maskf = persist.tile([P, 2, P], F32)
        nc.gpsimd.memset(maskf[:], 0.0)
        nc.gpsimd.memset(maskf[:, 0, 0:64], 1.0)
        nc.gpsimd.memset(maskf[:, 1, 64:128], 1.0)
        maskr = persist.tile([P, 2, P], F32R)
        nc.vector.tensor_copy(out=maskr[:], in_=maskf[:])

        rep_ctx = tc.For_i(0, reps, 1) if reps > 1 else None
        if rep_ctx is not None:
            stack.enter_context(rep_ctx)

        qkvT = persist.tile([P, ET, N], F32R)    # [dp, e-tile, n]; tiles 0-2=Q,3-5=K,6-8=V
        attnT = persist.tile([P, DT, N], F32R)   # unnormalized U^T, heads stacked
        vnat = persist.tile([P, NPAIR, NT, 130], F32R)  # [m-part, pair, m-tile, V_A|1|V_B|1]
        woutT = persist.tile([P, DT, D], F32R)
        bout_rep = persist.tile([P, D], F32)
        bqkv_sb = persist.tile([P, ET], F32)

        # ---------------- Stage A/B: loads + PE transposes ----------------
        with tc.tile_pool(name="early", bufs=1) as early:
            wqkvT = early.tile([P, DT, E3], F32R)
            xT = early.tile([P, DT, N], F32R)

            nc.sync.dma_start(out=bqkv_sb[:], in_=bqkv_ext[:].rearrange("(o p) -> p o", p=P))
            btmp = early.tile([1, D], F32)
            nc.sync.dma_start(out=btmp[:], in_=bout_ext[:].rearrange("d -> () d"))
            pbx = ps.tile([P, D], F32, tag="ps")
            nc.tensor.matmul(pbx[:], lhsT=onesf[0:1, :], rhs=btmp[0:1, :], start=True, stop=True)
            nc.vector.tensor_copy(out=bout_rep[:], in_=pbx[:])

            for eo in range(ET):
                wt = early.tile([P, D], F32, tag="wload", bufs=3)
                nc.sync.dma_start(out=wt[:], in_=wqkv_ext[eo * P:(eo + 1) * P, :])
                for do in range(DT):
                    pt = ps.tile([P, P], F32, tag="ps")
                    nc.tensor.transpose(pt[:], wt[:, do * P:(do + 1) * P], identity)
                    nc.vector.tensor_copy(out=wqkvT[:, do, eo * P:(eo + 1) * P], in_=pt[:])

            for eo in range(DT):
                wt = early.tile([P, D], F32, tag="wload", bufs=3)
                nc.sync.dma_start(out=wt[:], in_=wout_ext[eo * P:(eo + 1) * P, :])
                for do in range(DT):
                    pt = ps.tile([P, P], F32, tag="ps")
                    nc.tensor.transpose(pt[:], wt[:, do * P:(do + 1) * P], identity)
                    nc.vector.tensor_copy(out=woutT[:, do, eo * P:(eo + 1) * P], in_=pt[:])

            for no in range(NT):
                xt = early.tile([P, D], F32, tag="xload", bufs=3)
                nc.sync.dma_start(out=xt[:], in_=x_ext[no * P:(no + 1) * P, :])
                for do in range(DT):
                    pt = ps.tile([P, P], F32, tag="ps")
                    nc.tensor.transpose(pt[:], xt[:, do * P:(do + 1) * P], identity)
                    nc.vector.tensor_copy(out=xT[:, do, no * P:(no + 1) * P], in_=pt[:])

            # ---------------- Stage C: QKV projection ----------------
            for eo in range(ET):
                for c in range(NSC):
                    lo = c * SCH
                    pq = ps.tile([P, SCH], F32, tag="ps")
                    for do in range(DT):
                        nc.tensor.matmul(
                            pq[:],
                            lhsT=wqkvT[:, do, eo * P:(eo + 1) * P],
                            rhs=xT[:, do, lo:lo + SCH],
                            start=(do == 0),
                            stop=(do == DT - 1),
                        )
                    dst = qkvT[:, eo, lo:lo + SCH]
                    bcol = bqkv_sb[:, eo:eo + 1]
                    if eo < 3:  # Q: (x@Wq + bq) * SCALE
                        nc.vector.tensor_scalar(
                            dst, pq[:], bcol, SCALE,
                            mybir.AluOpType.add, mybir.AluOpType.mult,
                        )
                    else:
                        nc.vector.tensor_scalar_add(dst, pq[:], bcol)

        if debug:
            nc.sync.dma_start(out=dbg_qkvT[:], in_=qkvT[:].bitcast(F32))

        # ---------------- V natural layout (+ones cols) for AV ----------------
        nc.gpsimd.memset(vnat[:, :, :, 64:65].bitcast(F32), 1.0)
        nc.gpsimd.memset(vnat[:, :, :, 129:130].bitcast(F32), 1.0)
        for pr in range(NPAIR):
            for j in range(NT):
                pt = ps.tile([P, P], F32, tag="ps")
                nc.tensor.transpose(pt[:], qkvT[:, 6 + pr, j * P:(j + 1) * P].bitcast(F32), identity)
                nc.vector.tensor_copy(out=vnat[:, pr, j, 0:64], in_=pt[:, 0:64])
                nc.vector.tensor_copy(out=vnat[:, pr, j, 65:129], in_=pt[:, 64:128])

        if debug:
            nc.sync.dma_start(out=dbg_vnat[:], in_=vnat[:].bitcast(F32))

        # ---------------- Stage D: attention per head pair ----------------
        with tc.tile_pool(name="attnp", bufs=1) as attnp:
            for pr in range(NPAIR):
                recipP = attnp.tile([P, N], F32, tag="recipP", bufs=2)
                for c in range(NSC):
                    cs = slice(c * SCH, (c + 1) * SCH)
                    uA = psu.tile([P, SCH], F32, tag="psu")
                    uB = psu.tile([P, SCH], F32, tag="psu")

                    PIPE = 2  # S-matmuls run PIPE iterations ahead of exp/AV

                    def s_mms(j):
                        js = slice(j * P, (j + 1) * P)
                        sA = ps.tile([P, SCH], F32, tag="ps", name=f"sA_{j}")
                        sB = ps.tile([P, SCH], F32, tag="ps", name=f"sB_{j}")
                        nc.tensor.matmul(
                            sA[:], lhsT=qkvT[0:64, 3 + pr, js],
                            rhs=qkvT[0:64, pr, cs], start=True, stop=True,
                        )
                        nc.tensor.matmul(
                            sB[:], lhsT=qkvT[64:128, 3 + pr, js],
                            rhs=qkvT[64:128, pr, cs], start=True, stop=True,
                        )
                        return sA, sB

                    spipe = [s_mms(j) for j in range(PIPE)]
                    for j in range(NT):
                        if j + PIPE < NT:
                            spipe.append(s_mms(j + PIPE))
                        sA, sB = spipe.pop(0)
                        eA = attnp.tile([P, SCH], F32R, tag="exp", bufs=6)
                        eB = attnp.tile([P, SCH], F32R, tag="exp", bufs=6)
                        if "dvexp" in ablate:
                            nc.vector.tensor_copy(out=eA[:], in_=sA[:])
                            nc.vector.tensor_copy(out=eB[:], in_=sB[:])
                        else:
                            nc.scalar.activation(eA[:], sA[:], mybir.ActivationFunctionType.Exp)
                            nc.scalar.activation(eB[:], sB[:], mybir.ActivationFunctionType.Exp)
                        if debug and pr == 0 and c == 0 and j == 0:
                            nc.sync.dma_start(out=dbg_expA[:], in_=eA[:].bitcast(F32))
                        nc.tensor.matmul(
                            uA[0:65, :], lhsT=vnat[:, pr, j, 0:65], rhs=eA[:],
                            start=(j == 0), stop=(j == NT - 1),
                        )
                        nc.tensor.matmul(
                            uB[0:65, :], lhsT=vnat[:, pr, j, 65:130], rhs=eB[:],
                            start=(j == 0), stop=(j == NT - 1),
                        )
                    # U'^T rows: u[0:64]=U_head, u[64]=softmax denominator.
                    # DMA cannot read PSUM: stage through SBUF via DVE, then
                    # DMA places the head-B half at partitions 64-127.
                    stA = attnp.tile([P, SCH], F32R, tag="stU", bufs=3)
                    stB = attnp.tile([P, SCH], F32R, tag="stU", bufs=3)
                    nc.vector.tensor_copy(out=stA[0:65, :], in_=uA[0:65, :])
                    nc.vector.tensor_copy(out=stB[0:65, :], in_=uB[0:65, :])
                    if debug and pr == 0 and c == 0:
                        nc.sync.dma_start(out=dbg_uA[:], in_=stA[:].bitcast(F32))
                    nc.sync.dma_start(out=attnT[0:64, pr, cs], in_=stA[0:64, :])
                    nc.sync.dma_start(out=attnT[64:128, pr, cs], in_=stB[0:64, :])
                    # denominator rows -> all partitions via K=1 masked
                    # f32r matmuls accumulating into one [128, 512] bank
                    pbb = ps.tile([P, SCH], F32, tag="ps")
                    nc.tensor.matmul(pbb[:], lhsT=maskr[64:65, 0, :],
                                     rhs=stA[64:65, :], start=True, stop=False)
                    nc.tensor.matmul(pbb[:], lhsT=maskr[64:65, 1, :],
                                     rhs=stB[64:65, :], start=False, stop=True)
                    nc.vector.reciprocal(recipP[:, cs], pbb[:])
                nc.vector.tensor_mul(attnT[:, pr, :], attnT[:, pr, :], recipP[:])
                if debug and pr == 0:
                    nc.sync.dma_start(out=dbg_recip[:], in_=recipP[:])

            if debug:
                nc.sync.dma_start(out=dbg_attnT[:], in_=attnT[:].bitcast(F32))

            # ---------------- Stage E: output projection ----------------
            for i in range(NT):
                py = ps.tile([P, D], F32, tag="ps")
                for do in range(DT):
                    nc.tensor.matmul(
                        py[:],
                        lhsT=attnT[:, do, i * P:(i + 1) * P],
                        rhs=woutT[:, do, :],
                        start=(do == 0), stop=(do == DT - 1),
                    )
                yt = attnp.tile([P, D], F32, tag="yt", bufs=3)
                nc.vector.tensor_add(yt[:], py[:], bout_rep[:])
                nc.sync.dma_start(out=out_ext[i * P:(i + 1) * P, :], in_=yt[:])

    nc.finalize()
    return nc


_NC_CACHE = None


def _get_nc():
    global _NC_CACHE
    if _NC_CACHE is None:
        _NC_CACHE = build_module()
    return _NC_CACHE


def run(x, w_qkv, b_qkv, w_out, b_out, **spmd_kwargs):
    x = np.ascontiguousarray(np.asarray(x, dtype=np.float32))
    w_qkv = np.ascontiguousarray(np.asarray(w_qkv, dtype=np.float32))
    b_qkv = np.ascontiguousarray(np.asarray(b_qkv, dtype=np.float32))
    w_out = np.ascontiguousarray(np.asarray(w_out, dtype=np.float32))
    b_out = np.ascontiguousarray(np.asarray(b_out, dtype=np.float32))

    nc = _get_nc()
    core_ids = list(range(B))
    in_maps = [
        {"x": x[b], "w_qkv": w_qkv, "b_qkv": b_qkv, "w_out": w_out, "b_out": b_out}
        for b in range(B)
    ]
    res = run_bass_kernel_spmd(nc, in_maps, core_ids, **spmd_kwargs)
    out = np.stack([res.results[b]["out"] for b in range(B)], axis=0)
    return out, res


def kernel(x, w_qkv, b_qkv, w_out, b_out):
    out, _ = run(x, w_qkv, b_qkv, w_out, b_out)
    return out
